# revision 1
# baseline (speedup 1.0000x reference)
"""BitFeedForward Trainium2 kernel (BitNet b1.58 FFN: act-quant -> w1 -> gelu
-> act-quant -> w2), data-parallel over tokens across the NeuronCores.

Math notes:
- activation_quant: q = round(x * s), s = 127/clip(rowmax|x|,1e-5). |q|<=127 so
  quantized values are exactly representable in bf16; the matmul of int-valued
  bf16 against ternary bf16 accumulated in fp32 PSUM is EXACT.
- weight_quant: tern = clip(round(w*s1), -1, 1), s1 = 1/clip(mean|w|,1e-5).
  Computed as round(clamp(w*s1, +-1.49999988)) which is identical (incl. the
  round-half-even corner at |z|=1.5 after clip).
- round() is the fp32 magic-number trick (add/sub 1.5*2^23, RNE) == jnp.round.
- mean|w| is computed cooperatively (each core reduces 1/n_cores of each
  weight + tiny AllReduce). The reduction is blocked (256-wide blocks) to
  keep fp32 summation error ~1e-7 relative: ulp differences vs the reference
  mean flip ternary values for |w*s| straddling 0.5.

SBUF: pools are static stack allocations -> phase-scoped pools with shared
tags. PSUM: tags ps(2 banks, phase-1 matmul), tr(2, transposes),
pso(4, phase-2 accumulators as half-bank [128,256] slices, 2 tokens/bank).
"""

from contextlib import ExitStack

import numpy as np

import concourse.bass as bass
import concourse.bacc as bacc
import concourse.tile as tile
from concourse import mybir
from concourse.masks import make_identity

F32 = mybir.dt.float32
BF16 = mybir.dt.bfloat16
AX = mybir.AxisListType
OP = mybir.AluOpType
AF = mybir.ActivationFunctionType

MAGIC = 1.5 * 2**23  # fp32 round-to-nearest-even magic constant
CLIP = 1.49999988    # largest fp32 < 1.5
EPS = 1e-5
INV127 = 1.0 / 127.0


def build_kernel(T, D, H, n_cores, n_weight_elems=None, slice_den=None,
                 mock_cc=False, tg_mode=False, reps=1):
    """Build the per-core SPMD kernel.

    Per-core inputs: x [T,D], w1t [D,H] (=w1.T), w2t [H,D] (=w2.T),
    w1s [D,H/n] and w2s [H,D/n] (this core's slice for the mean-reduce).
    Output: out [T,D].
    """
    Tt = T // 128          # token tiles
    Dk = D // 128          # k-tiles of D (phase-1 contraction)
    HC = 512               # phase-1 H chunk (one PSUM bank of f32)
    NC1 = H // HC
    Hk = H // 128          # H k-tiles (phase-2 contraction)
    QC = min(2048, H)      # phase-1.5 read chunk
    NQ = H // QC
    SL = min(512, H)       # phase-2 w2 k-slab
    SLk = SL // 128
    NSL = H // SL
    DC = min(512, D)       # phase-2 D chunk (PSUM bank)
    ND = D // DC
    if slice_den is None:
        slice_den = n_cores
    HSn = H // slice_den
    DSn = D // slice_den
    if n_weight_elems is None:
        n_weight_elems = H * D

    nc = bacc.Bacc("TRN2", target_bir_lowering=False, debug=False,
                   num_devices=n_cores)

    x_ap = nc.dram_tensor("x", [T, D], F32, kind="ExternalInput").ap()
    w1t_ap = nc.dram_tensor("w1t", [D, H], F32, kind="ExternalInput").ap()
    w2t_ap = nc.dram_tensor("w2t", [H, D], F32, kind="ExternalInput").ap()
    w1s_ap = nc.dram_tensor("w1s", [D, HSn], F32, kind="ExternalInput").ap()
    w2s_ap = nc.dram_tensor("w2s", [H, DSn], F32, kind="ExternalInput").ap()
    out_ap = nc.dram_tensor("out", [T, D], F32, kind="ExternalOutput").ap()

    w1t_v = w1t_ap.rearrange("(k p) h -> p k h", p=128)      # [128, Dk, H]
    w2t_v = w2t_ap.rearrange("(s p) d -> p s d", p=128)      # [128, Hk, D]
    w1s_v = w1s_ap.rearrange("(k p) h -> p k h", p=128)
    w2s_v = w2s_ap.rearrange("(s p) d -> p s d", p=128)

    with tile.TileContext(nc) as tc:
        with ExitStack() as ctx:
            persist = ctx.enter_context(tc.tile_pool(name="persist", bufs=1))
            stage = ctx.enter_context(tc.tile_pool(name="stage", bufs=1))
            dram = ctx.enter_context(
                tc.tile_pool(name="dram", bufs=1, space="DRAM"))
            psum = ctx.enter_context(
                tc.tile_pool(name="psum", bufs=1, space="PSUM"))

            if tg_mode:
                def ps_mm():
                    return psum.tile([128, 512], F32, tag="ps", name="ps",
                                     bufs=2)

                def ps_tr():
                    return psum.tile([128, 512], F32, tag="tr", name="tr",
                                     bufs=2)

                def ps_out():
                    return psum.tile([128, 512], F32, tag="pso", name="pso",
                                     bufs=4)
            else:
                def ps_mm():
                    return psum.tile([128, 512], F32, tag="ps", name="ps",
                                     bufs=8)
                ps_tr = ps_mm
                ps_out = ps_mm

            def st8(nm):
                return stage.tile([128, 2048], F32, tag="st8", name=nm, bufs=3)

            def st2(nm):
                return stage.tile([128, 512], F32, tag="st2", name=nm, bufs=4)

            def stsm(nm):
                return stage.tile([128, 1], F32, tag="stsm", name=nm, bufs=4)

            # ---- constants ----
            ident = persist.tile([128, 128], F32, tag="ident")
            make_identity(nc, ident[:])
            magicv = persist.tile([128, 1], F32, tag="magicv")
            nc.gpsimd.memset(magicv[:], MAGIC)
            ones_col = persist.tile([128, 1], F32, tag="ones_col")
            nc.gpsimd.memset(ones_col[:], 1.0)
            ones_row = persist.tile([1, 128], F32, tag="ones_row")
            nc.gpsimd.memset(ones_row[:], 1.0)

            # cols per t: 0=sx 1=invsx 2=deq1 3=runmax 4=sh 5=invsh 6=deq2
            pertok = persist.tile([128, 8 * Tt], F32, tag="pertok")
            partials = persist.tile([128, 8], F32, tag="partials")
            red8 = persist.tile([1, 8], F32, tag="red8")
            tot2 = persist.tile([1, 2], F32, tag="tot2")
            cst = persist.tile([1, 16], F32, tag="cst")
            ccr = persist.tile([1, 16], F32, tag="ccr")
            vals = persist.tile([1, 4], F32, tag="vals")
            bcast = persist.tile([128, 4], F32, tag="bcast")

            hbuf = dram.tile([T, H], F32, tag="hbuf")
            w2qd = dram.tile([ND, NSL, 128, SLk * DC], BF16, tag="w2qd")
            ccin = dram.tile([1, 16], F32, tag="ccin")
            ccout = dram.tile([1, 16], F32, tag="ccout")

            S1 = bcast[:, 0:1]
            S2 = bcast[:, 1:2]
            M1W = bcast[:, 2:3]
            M2W = bcast[:, 3:4]

            def one_pass():
                with ExitStack() as ctxA:
                    pool_a = ctxA.enter_context(
                        tc.tile_pool(name="pool_a", bufs=1))
                    xqt = pool_a.tile([128, Tt * Dk * 128], BF16, tag="xqt")

                    # ---- phase 0b first: weight scale partials + AllReduce ----
                    # (emitted first so its DMAs and the collective start early)
                    for i, (src_v, nk, ncols) in enumerate(
                            ((w1s_v, Dk, HSn), (w2s_v, Hk, DSn))):
                        npiece = 4
                        if nk >= npiece:
                            assert nk % npiece == 0
                            pieces = [src_v[:, p * (nk // npiece):
                                            (p + 1) * (nk // npiece), :]
                                      for p in range(npiece)]
                        else:
                            assert ncols % npiece == 0
                            cs = ncols // npiece
                            pieces = [src_v[:, :, p * cs:(p + 1) * cs]
                                      for p in range(npiece)]
                        for p, piece in enumerate(pieces):
                            pk, pc = piece.shape[1], piece.shape[2]
                            nfree = pk * pc
                            wt = pool_a.tile([128, pk, pc],
                                             F32, tag="wf", name="p0", bufs=2)
                            nc.sync.dma_start(wt[:], piece)
                            # blocked two-level reduce for fp32 accuracy
                            nblk = max(1, nfree // 256)
                            l1 = stage.tile([128, nblk], F32, tag="l1",
                                            name="l1", bufs=2)
                            nc.vector.tensor_reduce(
                                l1[:],
                                wt[:].rearrange("p a b -> p (a b)")
                                     .rearrange("p (n b) -> p n b", n=nblk),
                                axis=AX.X, op=OP.add, apply_absolute_value=True)
                            nc.vector.tensor_reduce(
                                partials[:, 4 * i + p:4 * i + p + 1], l1[:],
                                axis=AX.X, op=OP.add)
                    pss = ps_mm()
                    nc.tensor.matmul(pss[0:1, 0:8], ones_col[:], partials[:],
                                     start=True, stop=True)
                    nc.scalar.copy(red8[:], pss[0:1, 0:8])
                    nc.vector.tensor_reduce(tot2[:, 0:1], red8[:, 0:4],
                                            axis=AX.X, op=OP.add)
                    nc.vector.tensor_reduce(tot2[:, 1:2], red8[:, 4:8],
                                            axis=AX.X, op=OP.add)
                    nc.gpsimd.memset(cst[:], 0.0)
                    nc.vector.tensor_copy(cst[:, 0:2], tot2[:])
                    nc.sync.dma_start(ccin[:], cst[:])
                    if mock_cc:
                        nc.sync.dma_start(ccout[:], ccin[:])
                    else:
                        nc.gpsimd.collective_compute(
                            "AllReduce", OP.add,
                            replica_groups=[list(range(n_cores))],
                            ins=[ccin.opt()], outs=[ccout.opt()])
                    nc.sync.dma_start(ccr[:], ccout[:])
                    nc.vector.tensor_scalar(vals[:, 2:4], ccr[:, 0:2],
                                            1.0 / float(n_weight_elems), EPS,
                                            OP.mult, OP.max)
                    nc.vector.reciprocal(vals[:, 0:2], vals[:, 2:4])
                    psb = ps_mm()
                    nc.tensor.matmul(psb[:, 0:4], ones_row[:], vals[:],
                                     start=True, stop=True)
                    nc.scalar.copy(bcast[:], psb[:, 0:4])

                    # ---- phase 0a: x load, scales, quantize, transpose ----
                    for t in range(Tt):
                        xt = st8("xt")
                        nc.sync.dma_start(xt[:, 0:D],
                                          x_ap[t * 128:(t + 1) * 128, :])
                        sx = pertok[:, 8 * t + 0:8 * t + 1]
                        invsx = pertok[:, 8 * t + 1:8 * t + 2]
                        mx = stsm("mx")
                        nc.vector.tensor_reduce(mx[:], xt[:, 0:D], axis=AX.X,
                                                op=OP.max,
                                                apply_absolute_value=True)
                        nc.vector.tensor_scalar(invsx, mx[:], EPS, INV127,
                                                OP.max, OP.mult)
                        nc.vector.reciprocal(sx, invsx)
                        qx = st8("qx")
                        nc.scalar.activation(qx[:, 0:D], xt[:, 0:D], AF.Identity,
                                             bias=magicv[:, 0:1], scale=sx)
                        for kk in range(0, Dk, 4):
                            kn = min(4, Dk - kk)
                            ps = ps_tr()
                            for k4 in range(kn):
                                k = kk + k4
                                nc.tensor.transpose(
                                    ps[:, k4 * 128:(k4 + 1) * 128],
                                    qx[:, k * 128:(k + 1) * 128], ident[:])
                            dst = xqt[:, (t * Dk + kk) * 128:
                                      (t * Dk + kk + kn) * 128]
                            nc.scalar.activation(dst, ps[:, 0:kn * 128],
                                                 AF.Copy, bias=-MAGIC)

                    # deq1_t = m1w * invsx_t
                    for t in range(Tt):
                        nc.vector.tensor_scalar(pertok[:, 8 * t + 2:8 * t + 3],
                                                pertok[:, 8 * t + 1:8 * t + 2],
                                                M1W, None, OP.mult)

                    # ---- phase 1: h = gelu(deq1 * (xq @ w1q^T)), rowmax ----
                    for hc in range(NC1):
                        w1f = pool_a.tile([128, Dk, HC], F32, tag="wf",
                                          name="w1f", bufs=2)
                        nc.sync.dma_start(
                            w1f[:], w1t_v[:, :, hc * HC:(hc + 1) * HC])
                        nc.gpsimd.tensor_scalar(w1f[:], w1f[:], S1, CLIP,
                                                OP.mult, OP.min)
                        nc.vector.tensor_scalar(w1f[:], w1f[:], -CLIP, MAGIC,
                                                OP.max, OP.add)
                        w1q = pool_a.tile([128, Dk, HC], BF16, tag="wq",
                                          name="w1q", bufs=2)
                        nc.vector.tensor_scalar(w1q[:], w1f[:], MAGIC, None,
                                                OP.subtract)
                        for t in range(Tt):
                            ps = ps_mm()
                            for k in range(Dk):
                                nc.tensor.matmul(
                                    ps[:, 0:HC],
                                    xqt[:, (t * Dk + k) * 128:
                                        (t * Dk + k) * 128 + 128],
                                    w1q[:, k, :],
                                    start=(k == 0), stop=(k == Dk - 1))
                            hsb = st2("hsb")
                            nc.scalar.activation(
                                hsb[:, 0:HC], ps[:, 0:HC], AF.Gelu,
                                scale=pertok[:, 8 * t + 2:8 * t + 3])
                            mx1 = stsm("mx1")
                            nc.vector.tensor_reduce(
                                mx1[:], hsb[:, 0:HC], axis=AX.X, op=OP.max,
                                apply_absolute_value=True)
                            runmax = pertok[:, 8 * t + 3:8 * t + 4]
                            if hc == 0:
                                nc.vector.tensor_copy(runmax, mx1[:])
                            else:
                                nc.vector.tensor_max(runmax, runmax, mx1[:])
                            nc.sync.dma_start(
                                hbuf[t * 128:(t + 1) * 128,
                                     hc * HC:(hc + 1) * HC], hsb[:, 0:HC])

                # ---- phases 1.5 + 2 share one scope (overlap enabled) ----
                with ExitStack() as ctxB:
                    pool_b = ctxB.enter_context(
                        tc.tile_pool(name="pool_b", bufs=1))
                    hqt = pool_b.tile([128, Hk * Tt * 128], BF16, tag="hqt")

                    # phase 1.5: quantize h, transpose into hqt
                    ncopy = 0
                    for t in range(Tt):
                        sh = pertok[:, 8 * t + 4:8 * t + 5]
                        invsh = pertok[:, 8 * t + 5:8 * t + 6]
                        deq2 = pertok[:, 8 * t + 6:8 * t + 7]
                        runmax = pertok[:, 8 * t + 3:8 * t + 4]
                        nc.vector.tensor_scalar(invsh, runmax, EPS, INV127,
                                                OP.max, OP.mult)
                        nc.vector.reciprocal(sh, invsh)
                        nc.vector.tensor_scalar(deq2, invsh, M2W, None, OP.mult)
                    for qc in range(NQ):
                        for t in range(Tt):
                            sh = pertok[:, 8 * t + 4:8 * t + 5]
                            hrd = st8("hrd")
                            nc.sync.dma_start(
                                hrd[:, 0:QC], hbuf[t * 128:(t + 1) * 128,
                                                   qc * QC:(qc + 1) * QC])
                            qtl = st8("qtl")
                            nc.scalar.activation(qtl[:, 0:QC], hrd[:, 0:QC],
                                                 AF.Identity,
                                                 bias=magicv[:, 0:1], scale=sh)
                            for jj in range(0, QC // 128, 4):
                                jn = min(4, QC // 128 - jj)
                                ps = ps_tr()
                                for j4 in range(jn):
                                    j = jj + j4
                                    nc.tensor.transpose(
                                        ps[:, j4 * 128:(j4 + 1) * 128],
                                        qtl[:, j * 128:(j + 1) * 128], ident[:])
                                k2a = qc * (QC // 128) + jj
                                # strided copy covering the jn transposed blocks
                                span = hqt[:, (k2a * Tt + t) * 128:
                                           (((k2a + jn - 1) * Tt + t) + 1) * 128]
                                dst = span.rearrange("p (n x) -> p n x",
                                                     x=128)[:, ::Tt, :]
                                src = ps[:, 0:jn * 128].rearrange(
                                    "p (n x) -> p n x", x=128)
                                if False:
                                    nc.scalar.activation(dst, src, AF.Copy,
                                                         bias=-MAGIC)
                                else:
                                    nc.vector.tensor_scalar(dst, src, MAGIC, None,
                                                            OP.subtract)
                                ncopy += 1

                    # phase 2: out = deq2 * (hq @ w2q^T), token groups of 4.
                    # Group 0 streams+ternarizes w2 (and caches w2q bf16 in DRAM);
                    # later groups re-read the cached bf16 (no recompute).
                    TG = min(4, Tt) if tg_mode else Tt
                    ngrp = (Tt + TG - 1) // TG
                    for dc in range(ND):
                        for g in range(ngrp):
                            toks = list(range(g * TG, min((g + 1) * TG, Tt)))
                            psob = {t: ps_out() for t in toks}
                            for sl in range(NSL):
                                w2q = pool_b.tile([128, SLk, DC], BF16, tag="w2q",
                                                  name="w2q", bufs=2)
                                if g == 0:
                                    w2f = pool_b.tile([128, SLk, DC], F32,
                                                      tag="w2f", name="w2f",
                                                      bufs=2)
                                    nc.sync.dma_start(
                                        w2f[:],
                                        w2t_v[:, sl * SLk:(sl + 1) * SLk,
                                              dc * DC:(dc + 1) * DC])
                                    nc.gpsimd.tensor_scalar(w2f[:], w2f[:], S2,
                                                            CLIP, OP.mult, OP.min)
                                    nc.vector.tensor_scalar(w2f[:], w2f[:], -CLIP,
                                                            MAGIC, OP.max, OP.add)
                                    nc.vector.tensor_scalar(w2q[:], w2f[:], MAGIC,
                                                            None, OP.subtract)
                                    if ngrp > 1:
                                        nc.sync.dma_start(
                                            w2qd[dc, sl],
                                            w2q[:].rearrange("p a b -> p (a b)"))
                                else:
                                    nc.sync.dma_start(
                                        w2q[:].rearrange("p a b -> p (a b)"),
                                        w2qd[dc, sl])
                                for t in toks:
                                    for kk in range(SLk):
                                        k2 = sl * SLk + kk
                                        nc.tensor.matmul(
                                            psob[t][:, 0:DC],
                                            hqt[:, (k2 * Tt + t) * 128:
                                                (k2 * Tt + t) * 128 + 128],
                                            w2q[:, kk, :],
                                            start=(k2 == 0), stop=(k2 == Hk - 1),
                                            skip_group_check=True)
                            for t in toks:
                                osb = st2("osb")
                                nc.scalar.activation(
                                    osb[:, 0:DC], psob[t][:, 0:DC], AF.Copy,
                                    scale=pertok[:, 8 * t + 6:8 * t + 7])
                                nc.sync.dma_start(
                                    out_ap[t * 128:(t + 1) * 128,
                                           dc * DC:(dc + 1) * DC], osb[:, 0:DC])


            for _rep in range(reps):
                one_pass()

    nc.compile()
    return nc


def shard_inputs(x, w1, w2, n_cores):
    """Host-side sharding: token shards + transposed weights + mean slices."""
    B, S, Dx = x.shape
    T_total = B * S
    T = T_total // n_cores
    xf = np.ascontiguousarray(x.reshape(T_total, Dx))
    w1t = np.ascontiguousarray(w1.T)  # [D, H]
    w2t = np.ascontiguousarray(w2.T)  # [H, D]
    H = w1.shape[0]
    D = Dx
    HSn = H // n_cores
    DSn = D // n_cores
    in_maps = []
    for i in range(n_cores):
        in_maps.append({
            "x": np.ascontiguousarray(xf[i * T:(i + 1) * T]),
            "w1t": w1t,
            "w2t": w2t,
            "w1s": np.ascontiguousarray(w1t[:, i * HSn:(i + 1) * HSn]),
            "w2s": np.ascontiguousarray(w2t[:, i * DSn:(i + 1) * DSn]),
        })
    return in_maps, (B, S, D, H, T)


# ---------------------------------------------------------------------------
# Self-contained entry point for grading: kernel(**inputs) -> np.ndarray
# ---------------------------------------------------------------------------
from concourse.bass_utils import run_bass_kernel_spmd

N_CORES = 8
B_, S_, D_, H_ = 4, 2048, 2048, 8192
T_ = (B_ * S_) // N_CORES  # tokens per core

_NC_CACHE = {}


def _get_nc():
    key = (T_, D_, H_, N_CORES)
    if key not in _NC_CACHE:
        _NC_CACHE[key] = build_kernel(T_, D_, H_, N_CORES)
    return _NC_CACHE[key]


def run_spmd(x, w1, w2, **run_kwargs):
    """Shard, run on the 8 cores, gather. Returns (out, BassKernelResults)."""
    x = np.asarray(x, dtype=np.float32)
    w1 = np.asarray(w1, dtype=np.float32)
    w2 = np.asarray(w2, dtype=np.float32)
    B, S, D = x.shape
    nc = _get_nc()
    in_maps, _meta = shard_inputs(x, w1, w2, N_CORES)
    res = run_bass_kernel_spmd(nc, in_maps, list(range(N_CORES)), **run_kwargs)
    outs = [res.results[i]["out"] for i in range(N_CORES)]
    out = np.concatenate(outs, axis=0).reshape(B, S, D).astype(np.float32)
    return out, res


def kernel(x, w1, w2):
    out, _ = run_spmd(x, w1, w2)
    return out

